# revision 6
# baseline (speedup 1.0000x reference)
"""Cross-attention (B=4, T=S=1024, C=1024, H=16, D=64) on 8 trn2 NeuronCores.

Collective-free fp16 pipeline. Core c handles batch b=c//2, query half hf=c%2
(512 q rows); every core recomputes full-S k/v for its batch — ~30us of extra
PE work instead of the pairwise AllGather the v1 kernel used (~225us in the
collective cost model, mostly unoverlapped).

Schedule (per-engine program order): k proj+RMS -> v proj heads 0-7 ->
q proj+RMS -> head pairs 0..3 with v proj heads 8-15 interleaved (PE work
fills the elementwise-bound pair cadence) -> pairs 4..7 -> output projection
(emitted before the attn_mean epilogue so PE overlaps the elementwise tail).

Everything on-chip is fp16 (PSUM f32): W/x/enc are cast on the host; rel err
vs the f32 reference is ~4e-3 (gate 2e-2). Per head pair (one q/k m-tile):
 - both heads' K=64 scores land in one [128, 1024] 2-bank PSUM tile (two
   N=512 matmuls; N>512 fails the real ISA check) -> ONE wide ACT exp
   exp(score/8 - 10) in fp16; the -10 keeps e^x in fp16 range and cancels.
 - y_aug[65, t] = v_aug^T @ exp accumulates over s-tiles; v_aug has a 16.0
   column so recip(y_aug[64]) = 1/(16 Z) directly.
 - rb broadcast per head via K=1 ones-row matmul + ACT copy, emitted per-head
   so head A's chain hides under head B's y matmuls.
 - yT = 16 * y_aug * rb via DVE scalar_tensor_tensor.
 - attn_mean acc[j] += exp * rb as [128, 1024] wide mul+add pairs, split
   DVE (j<6) / GpSimd (j>=6); acc keeps even|odd head lanes, folded and
   DMA'd out at the end. GpSimd never touches PSUM (HW restriction).
Host transposes/casts inputs, reassembles fp16 outputs to f32.
"""

import numpy as np

import concourse.bacc as bacc
import concourse.mybir as mybir
import concourse.tile as tile
from concourse.bass_utils import run_bass_kernel_spmd

F32 = mybir.dt.float32
F16 = mybir.dt.float16
AF = mybir.ActivationFunctionType
ALU = mybir.AluOpType

B, T, S, C, H = 4, 1024, 1024, 1024, 16
D = C // H            # 64
TN = 512              # per-core q rows
KT = 8                # contraction tiles (C/128)
MT = 8                # output-channel tiles
ST = 8                # s tiles (S/128)
DA = D + 1            # augmented head width (16.0 column at 64)
EB = -10.0            # exp bias: exp(score/8 + EB), cancels in softmax


def build(stt_pool=False, dve_j=6, qf1_dve=False):
    nc = bacc.Bacc("TRN2", target_bir_lowering=False, debug=False, num_devices=8)

    xT_d = nc.dram_tensor("xT", [C, TN], F16, kind="ExternalInput")
    encT_d = nc.dram_tensor("encT", [C, S], F16, kind="ExternalInput")
    wq_d = nc.dram_tensor("wq", [C, C], F16, kind="ExternalInput")
    wk_d = nc.dram_tensor("wk", [C, C], F16, kind="ExternalInput")
    wv_d = nc.dram_tensor("wv", [C, C], F16, kind="ExternalInput")
    wp_d = nc.dram_tensor("wp", [C, C], F16, kind="ExternalInput")
    bq_d = nc.dram_tensor("bq", [128, MT], F32, kind="ExternalInput")
    bk_d = nc.dram_tensor("bk", [128, MT], F32, kind="ExternalInput")
    bp_r_d = nc.dram_tensor("bp_r", [C], F16, kind="ExternalInput")
    bv_d = nc.dram_tensor("bv", [C], F16, kind="ExternalInput")
    qs_d = nc.dram_tensor("qs", [C], F16, kind="ExternalInput")
    ks_d = nc.dram_tensor("ks", [C], F16, kind="ExternalInput")

    yT_o = nc.dram_tensor("youtT", [C, TN], F16, kind="ExternalOutput")
    am_o = nc.dram_tensor("ameanT", [S, TN], F16, kind="ExternalOutput")

    def wide_load(w_sb, w_d, chunks=1):
        ap = w_d.ap().rearrange("(k p) c -> p k c", p=128)
        kc = KT // chunks
        for i in range(chunks):
            nc.sync.dma_start(
                w_sb[:, i * kc : (i + 1) * kc, :], ap[:, i * kc : (i + 1) * kc, :]
            )

    with tile.TileContext(nc) as tc:
        with (
            tc.tile_pool(name="const", bufs=1) as cst,
            tc.tile_pool(name="qt", bufs=1) as qt_pool,
            tc.tile_pool(name="kt", bufs=1) as kt_pool,
            tc.tile_pool(name="vt", bufs=1) as vt_pool,
            tc.tile_pool(name="acc", bufs=1) as acc_pool,
            tc.tile_pool(name="yt", bufs=1) as yt_pool,
            tc.tile_pool(name="w", bufs=2) as w_pool,
            tc.tile_pool(name="inx", bufs=1) as in_pool,
        ):
            # ---- first-issue DMAs: k-projection inputs (2 chunks each) ----
            wk_sb = w_pool.tile([128, KT, C], F16, tag="W", name="wk_sb")
            eT = in_pool.tile([128, KT, S], F16, name="eT")
            eap = encT_d.ap().rearrange("(k p) s -> p k s", p=128)
            nc.scalar.dma_start(eT[:, 0:4, :], eap[:, 0:4, :])
            wide_load(wk_sb, wk_d, chunks=2)
            nc.scalar.dma_start(eT[:, 4:8, :], eap[:, 4:8, :])

            # ---- constants ----
            ones_col_f = cst.tile([128, 1], F32)
            nc.vector.memset(ones_col_f[:], 1.0)
            ones_col = cst.tile([128, 1], F16)
            nc.vector.tensor_copy(ones_col[:], ones_col_f[:])
            ones_row_f = cst.tile([1, 512], F32)
            nc.vector.memset(ones_row_f[:], 1.0)
            ones_wrow = cst.tile([1, 512], F16)
            nc.vector.tensor_copy(ones_wrow[:], ones_row_f[:])
            ones_row = ones_wrow[:, 0:128]
            eps_t = cst.tile([1, 1], F32)
            nc.vector.memset(eps_t[:], 1e-6)
            ebias_t = cst.tile([128, 1], F32)
            nc.vector.memset(ebias_t[:], EB)

            bq_sb = cst.tile([128, MT], F32)
            bk_sb = cst.tile([128, MT], F32)
            nc.sync.dma_start(bk_sb[:], bk_d.ap())
            nc.sync.dma_start(bq_sb[:], bq_d.ap())
            bv_sb = cst.tile([1, C], F16)
            qs_sb = cst.tile([1, C], F16)
            ks_sb = cst.tile([1, C], F16)
            bp_row = cst.tile([1, C], F16)
            nc.sync.dma_start(bv_sb[:], bv_d.ap().unsqueeze(0))
            nc.sync.dma_start(ks_sb[:], ks_d.ap().unsqueeze(0))
            nc.sync.dma_start(qs_sb[:], qs_d.ap().unsqueeze(0))
            nc.sync.dma_start(bp_row[:], bp_r_d.ap().unsqueeze(0))

            xT = in_pool.tile([128, KT, TN], F16, name="xTs")
            nc.sync.dma_start(xT[:], xT_d.ap().rearrange("(k p) t -> p k t", p=128))

            qT = [qt_pool.tile([128, TN], F16, tag=f"qT{m}", name=f"qT{m}") for m in range(MT)]
            kT = [kt_pool.tile([128, S], F16, tag=f"kT{m}", name=f"kT{m}") for m in range(MT)]
            vf = [vt_pool.tile([128, H, DA], F16, tag=f"vf{j}", name=f"vf{j}") for j in range(ST)]
            # wide accumulators: cols 0:512 even heads, 512:1024 odd heads
            acc = [acc_pool.tile([128, 2 * TN], F16, tag=f"acc{j}", name=f"acc{j}") for j in range(ST)]
            yT = [yt_pool.tile([128, TN], F16, tag=f"yT{k}", name=f"yT{k}") for k in range(KT)]

            # y_aug denominator column: 16.0 so 1/y_aug[64] = 1/(16 Z)
            for j in range(ST):
                nc.vector.memset(vf[j][:, :, D], 16.0)

            # ============ k projection + RMS (full S, wide N=1024) ============
            with (
                tc.tile_pool(name="pk_sq", bufs=3) as pk_sq,
                tc.tile_pool(name="pk_ps", bufs=2, space="PSUM") as pk_ps,
                tc.tile_pool(name="pk_ss", bufs=1, space="PSUM") as pk_ss,
                tc.tile_pool(name="pk_f", bufs=1, space="PSUM") as pk_f,
            ):
                ssum = [pk_ss.tile([1, 512], F32, tag=f"ssum{sh}", name=f"kss{sh}")
                        for sh in range(2)]
                for m in range(MT):
                    ps = pk_ps.tile([128, S], F32, tag="proj")
                    for sh in range(2):
                        for k in range(KT):
                            nc.tensor.matmul(
                                ps[:, sh * 512 : (sh + 1) * 512],
                                wk_sb[:, k, m * 128 : (m + 1) * 128],
                                eT[:, k, sh * 512 : (sh + 1) * 512],
                                start=(k == 0),
                                stop=(k == KT - 1),
                            )
                    nc.vector.tensor_scalar_add(kT[m][:], ps[:], bk_sb[:, m : m + 1])
                    sq = pk_sq.tile([128, S], F16, tag="sq")
                    nc.vector.tensor_mul(sq[:], kT[m][:], kT[m][:])
                    for sh in range(2):
                        nc.tensor.matmul(
                            ssum[sh][:], ones_col[:],
                            sq[:, sh * 512 : (sh + 1) * 512],
                            start=(m == 0), stop=(m == MT - 1),
                        )
                krr = pk_sq.tile([1, S], F16, tag="krr", name="krr")
                for sh in range(2):
                    rms = pk_sq.tile([1, 512], F32, tag="rms")
                    nc.scalar.activation(
                        rms[:], ssum[sh][:], AF.Sqrt, scale=1.0 / C, bias=eps_t[:]
                    )
                    with nc.allow_low_precision(reason="rms rsqrt broadcast"):
                        nc.vector.reciprocal(krr[:, sh * 512 : (sh + 1) * 512], rms[:])
                for m in range(MT):
                    fps = pk_f.tile([128, S], F32, tag="fps")
                    for sh in range(2):
                        nc.tensor.matmul(
                            fps[:, sh * 512 : (sh + 1) * 512],
                            ks_sb[:, m * 128 : (m + 1) * 128],
                            krr[:, sh * 512 : (sh + 1) * 512],
                            start=True, stop=True,
                        )
                    f1 = pk_sq.tile([128, S], F16, tag="f1")
                    nc.scalar.activation(f1[:], fps[:], AF.Copy, bias=1.0)
                    nc.vector.tensor_mul(kT[m][:], kT[m][:], f1[:])

            # v projection for one head half (heads jh*8 .. jh*8+7), all s
            def v_proj(pv_ps, wv_sb, jh, sts):
                for st in sts:
                    ps = pv_ps.tile([128, 512], F32, tag="proj")
                    for k in range(KT):
                        nc.tensor.matmul(
                            ps[:],
                            eT[:, k, st * 128 : (st + 1) * 128],
                            wv_sb[:, k, jh * 512 : (jh + 1) * 512],
                            start=(k == 0),
                            stop=False,
                        )
                    nc.tensor.matmul(
                        ps[:], ones_row[:], bv_sb[:, jh * 512 : (jh + 1) * 512],
                        start=False, stop=True,
                    )
                    nc.scalar.activation(
                        vf[st][:, jh * 8 : (jh + 1) * 8, 0:D],
                        ps[:].rearrange("p (h d) -> p h d", h=8),
                        AF.Copy,
                    )

            # ============ v projection heads 0-7 ============
            wv_sb = w_pool.tile([128, KT, C], F16, tag="W", name="wv_sb")
            with (
                tc.tile_pool(name="pv0_ps", bufs=2, space="PSUM") as pv0_ps,
            ):
                wide_load(wv_sb, wv_d, chunks=2)
                v_proj(pv0_ps, wv_sb, 0, range(ST))

            # ============ q projection + RMS (TN rows) ============
            with (
                tc.tile_pool(name="pq_sq", bufs=3) as pq_sq,
                tc.tile_pool(name="pq_ps", bufs=2, space="PSUM") as pq_ps,
                tc.tile_pool(name="pq_ss", bufs=1, space="PSUM") as pq_ss,
            ):
                wq_sb = w_pool.tile([128, KT, C], F16, tag="W", name="wq_sb")
                wide_load(wq_sb, wq_d, chunks=2)
                qss = pq_ss.tile([1, TN], F32, tag="qss", name="qss")
                for m in range(MT):
                    ps = pq_ps.tile([128, TN], F32, tag="proj")
                    for k in range(KT):
                        nc.tensor.matmul(
                            ps[:],
                            wq_sb[:, k, m * 128 : (m + 1) * 128],
                            xT[:, k, :],
                            start=(k == 0),
                            stop=(k == KT - 1),
                        )
                    nc.vector.tensor_scalar_add(qT[m][:], ps[:], bq_sb[:, m : m + 1])
                    sq = pq_sq.tile([128, TN], F16, tag="sq")
                    nc.vector.tensor_mul(sq[:], qT[m][:], qT[m][:])
                    nc.tensor.matmul(
                        qss[:], ones_col[:], sq[:],
                        start=(m == 0), stop=(m == MT - 1),
                    )
                qrms = pq_sq.tile([1, TN], F32, tag="qrms", name="qrms")
                nc.scalar.activation(
                    qrms[:], qss[:], AF.Sqrt, scale=1.0 / C, bias=eps_t[:]
                )
                qrr = pq_sq.tile([1, TN], F16, tag="qrr", name="qrr")
                with nc.allow_low_precision(reason="rms rsqrt broadcast"):
                    nc.vector.reciprocal(qrr[:], qrms[:])
                for m in range(MT):
                    fps = pq_ps.tile([128, TN], F32, tag="proj")
                    nc.tensor.matmul(
                        fps[:], qs_sb[:, m * 128 : (m + 1) * 128], qrr[:],
                        start=True, stop=True,
                    )
                    f1 = pq_sq.tile([128, TN], F16, tag="qf1")
                    if qf1_dve:
                        nc.vector.tensor_scalar_add(f1[:], fps[:], 1.0)
                    else:
                        nc.scalar.activation(f1[:], fps[:], AF.Copy, bias=1.0)
                    nc.vector.tensor_mul(qT[m][:], qT[m][:], f1[:])

            # ---- attention (pairs 0-3 interleave v-proj heads 8-15) ----
            if True:
                with (
                    tc.tile_pool(name="p3_e", bufs=2) as p3_e,
                    tc.tile_pool(name="p3_rb", bufs=2) as p3_rb,
                    tc.tile_pool(name="p3_t", bufs=3) as p3_t,
                    tc.tile_pool(name="pv_ps", bufs=2, space="PSUM") as pv_ps,
                    tc.tile_pool(name="p3_sc", bufs=2, space="PSUM") as p3_sc,
                    tc.tile_pool(name="p3_y", bufs=2, space="PSUM") as p3_y,
                ):
                    wp_sb = w_pool.tile([128, KT, C], F16, tag="W", name="wp_sb")
                    wide_load(wp_sb, wp_d)

                    def attn_pair(p):
                        mt = p
                        heads = [(2 * p, 0), (2 * p + 1, 64)]
                        exps = []
                        for j in range(ST):
                            sc = p3_sc.tile([128, 2 * TN], F32, tag="sc")
                            for h, base in heads:
                                nc.tensor.matmul(
                                    sc[:, base * 8 : base * 8 + TN],
                                    kT[mt][base : base + 64, j * 128 : (j + 1) * 128],
                                    qT[mt][base : base + 64, :],
                                    start=True,
                                    stop=True,
                                )
                            ex = p3_e.tile([128, 2 * TN], F16, tag=f"exp{j}")
                            nc.scalar.activation(
                                ex[:], sc[:], AF.Exp, scale=float(D) ** -0.5,
                                bias=ebias_t[:],
                            )
                            exps.append(ex)
                        recip = p3_rb.tile([1, 2 * TN], F16, tag="recip")
                        rb_sb = p3_rb.tile([128, 2 * TN], F16, tag="rbsb")
                        y_pss = []
                        for h, base in heads:
                            y_ps = p3_y.tile([DA, TN], F32, tag="y")
                            y_pss.append(y_ps)
                            for j in range(ST):
                                nc.tensor.matmul(
                                    y_ps[:],
                                    vf[j][:, h, :],
                                    exps[j][:, base * 8 : base * 8 + TN],
                                    start=(j == 0),
                                    stop=(j == ST - 1),
                                )
                            # per-head rb = 1/(16 Z) broadcast: head A's chain
                            # completes while head B's y matmuls run
                            with nc.allow_low_precision(reason="softmax 1/Z bcast"):
                                nc.vector.reciprocal(
                                    recip[:, base * 8 : base * 8 + TN], y_ps[64:65, :]
                                )
                            rb_ps = p3_sc.tile([128, 2 * TN], F32, tag="sc")
                            nc.tensor.matmul(
                                rb_ps[:, 0:TN],
                                ones_row[:],
                                recip[:, base * 8 : base * 8 + TN],
                                start=True, stop=True,
                            )
                            nc.scalar.activation(
                                rb_sb[:, base * 8 : base * 8 + TN],
                                rb_ps[:, 0:TN], AF.Copy,
                            )
                        # yT rows = 16 * y_aug * rb
                        for (h, base), y_ps in zip(heads, y_pss):
                            (nc.gpsimd if stt_pool else nc.vector).scalar_tensor_tensor(
                                yT[mt][base : base + 64, :],
                                y_ps[0:64, :],
                                16.0,
                                rb_sb[0:64, base * 8 : base * 8 + TN],
                                ALU.mult,
                                ALU.mult,
                            )
                        # attn_mean: acc[j] += exp[j] * rb (wide, both heads)
                        for j in range(ST):
                            eng = nc.vector if j < dve_j else nc.gpsimd
                            if p == 0:
                                eng.tensor_mul(acc[j][:], exps[j][:], rb_sb[:])
                            else:
                                t = p3_t.tile([128, 2 * TN], F16, tag=f"t{j % 3}")
                                eng.tensor_mul(t[:], exps[j][:], rb_sb[:])
                                eng.tensor_add(acc[j][:], acc[j][:], t[:])

                    for p in range(4):
                        attn_pair(p)
                        v_proj(pv_ps, wv_sb, 1, range(2 * p, 2 * p + 2))
                    for p in range(4, 8):
                        attn_pair(p)

                    # output projection (only needs yT; emitted before the
                    # attn_mean epilogue so PE overlaps the elementwise tail;
                    # bias via K=1 ones matmul + ACT copy keeps DVE free)
                    for m in range(MT):
                        ps = pv_ps.tile([128, TN], F32, tag="proj")
                        for k in range(KT):
                            nc.tensor.matmul(
                                ps[:],
                                wp_sb[:, k, m * 128 : (m + 1) * 128],
                                yT[k][:],
                                start=(k == 0),
                                stop=False,
                            )
                        nc.tensor.matmul(
                            ps[:],
                            bp_row[:, m * 128 : (m + 1) * 128],
                            ones_wrow[:],
                            start=False,
                            stop=True,
                        )
                        yo = p3_t.tile([128, TN], F16, tag=f"yo{m % 2}")
                        nc.scalar.activation(yo[:], ps[:], AF.Copy)
                        nc.sync.dma_start(yT_o.ap()[m * 128 : (m + 1) * 128, :], yo[:])

                    for j in range(ST):
                        amo = p3_t.tile([128, TN], F16, tag=f"amo{j % 2}")
                        eng = nc.gpsimd if j % 2 else nc.vector
                        eng.tensor_add(amo[:], acc[j][:, 0:TN], acc[j][:, TN:])
                        nc.sync.dma_start(
                            am_o.ap()[j * 128 : (j + 1) * 128, :], amo[:]
                        )

    nc.compile()
    return nc


_NC_CACHE = None


def _get_nc():
    global _NC_CACHE
    if _NC_CACHE is None:
        _NC_CACHE = build()
    return _NC_CACHE


def make_in_maps(x, encoder_output, Wq, bq, Wk, bk, Wv, bv, q_scale, k_scale,
                 Wp, bp):
    f16 = np.float16
    x = np.asarray(x, np.float32)
    enc = np.asarray(encoder_output, np.float32)
    Wq = np.ascontiguousarray(np.asarray(Wq, f16))
    Wk = np.ascontiguousarray(np.asarray(Wk, f16))
    Wv = np.ascontiguousarray(np.asarray(Wv, f16))
    Wp = np.ascontiguousarray(np.asarray(Wp, f16))
    bq_t = np.ascontiguousarray(np.asarray(bq, np.float32).reshape(MT, 128).T)
    bk_t = np.ascontiguousarray(np.asarray(bk, np.float32).reshape(MT, 128).T)
    bp_r = np.ascontiguousarray(np.asarray(bp, f16))
    bv = np.ascontiguousarray(np.asarray(bv, f16))
    qs = np.ascontiguousarray(np.asarray(q_scale, f16))
    ks = np.ascontiguousarray(np.asarray(k_scale, f16))

    in_maps = []
    for c in range(8):
        b, hf = c // 2, c % 2
        xT = np.ascontiguousarray(x[b, hf * TN : (hf + 1) * TN, :].T.astype(f16))
        encT = np.ascontiguousarray(enc[b].T.astype(f16))
        in_maps.append(
            dict(xT=xT, encT=encT, wq=Wq, wk=Wk, wv=Wv, wp=Wp,
                 bq=bq_t, bk=bk_t, bp_r=bp_r, bv=bv, qs=qs, ks=ks)
        )
    return in_maps


def kernel(x, encoder_output, Wq, bq, Wk, bk, Wv, bv, q_scale, k_scale, Wp, bp,
           _trace=False):
    in_maps = make_in_maps(x, encoder_output, Wq, bq, Wk, bk, Wv, bv, q_scale,
                           k_scale, Wp, bp)
    nc = _get_nc()
    res = run_bass_kernel_spmd(nc, in_maps, core_ids=list(range(8)), trace=_trace)

    y = np.empty((B, T, C), np.float32)
    amean = np.empty((B, T, S), np.float32)
    for c in range(8):
        b, hf = c // 2, c % 2
        r = res.results[c]
        y[b, hf * TN : (hf + 1) * TN, :] = r["youtT"].T.astype(np.float32)
        amean[b, hf * TN : (hf + 1) * TN, :] = r["ameanT"].T.astype(np.float32)
    if _trace:
        kernel.last_exec_time_ns = res.exec_time_ns
        kernel.last_results = res
    return y, amean
